# revision 1
# baseline (speedup 1.0000x reference)
"""3D Canny edge detector on 8 Trainium2 cores.

Shard D=256 across 8 cores (32 output slices each) with a 4-voxel halo,
entirely host-side (no collectives). Per-core layout: partitions =
3 h-strips x 40 local d-slices (120 of 128), free dim = (94 h-rows, 52 w-cols)
per w-tile. All three stencil axes are then partition- or free-dim shifts.
sqrt is eliminated by comparing squared magnitudes against squared thresholds;
the Gaussian is applied unnormalized ([u,1,u] per axis) with the normalization
folded into the thresholds. Global-border zeroing of the gradient magnitude is
done via a per-partition mask input (d borders, differs per core) fused into
the ScalarE square ops, plus tiny memsets for the h/w border rows/cols.
"""
import json
import numpy as np

import concourse.bass as bass
import concourse.mybir as mybir
from concourse.bass_utils import run_bass_kernel_spmd
from concourse.tile import TileContext

F32 = mybir.dt.float32
I8 = mybir.dt.int8
AL = mybir.AluOpType
SQ = mybir.ActivationFunctionType.Square

N_CORES = 8
D, H, W = 256, 256, 256
DLOC = 40           # 32 output slices + 4 halo each side
NPART = 120         # 3 strips * 40
ROWS = 94           # h rows per strip tile (out rows + up to 4 halo each side)
COLS = 52           # w cols per tile (44 out + 4 halo each side)
WT_OUT = 44
N_WT = 6
STRIP_OFF = (0, 85, 170)                       # padded-h offset per strip
STRIP_OUT = ((4, 86, 0), (5, 85, 86), (5, 85, 171))  # (first r, n rows, h0)

U = float(np.exp(np.float64(-0.5)))
SC = (1.0 + 2.0 * U) ** 3
HI2 = float((0.2 * SC) ** 2)
LO2 = float((0.1 * SC) ** 2)


def _fix_bir_json_bytes(raw: bytes) -> bytes:
    """walrus codegen has per-instruction sync-wait-slot limits (1 for CTRL
    Drain, 2 for compute structs). Hoist excess waits onto prepended
    single-wait Drain instructions on the same engine."""
    m = json.loads(raw)
    changed = False
    for fn in m.get("functions", []):
        for bb in fn.get("blocks", []):
            out = []
            for inst in bb.get("instructions", []):
                si = inst.get("sync_info") or {}
                waits = si.get("on_wait") or []
                lim = 1
                if len(waits) > lim and inst.get("engine") not in (None, "Unassigned"):
                    changed = True
                    keep_n = lim
                    for i, wt in enumerate(waits[:-keep_n] if keep_n else waits):
                        out.append({
                            "debug": inst.get("debug", 0),
                            "engine": inst["engine"],
                            "ins": [], "outs": [],
                            "is_reset_sema": False,
                            "name": f"{inst['name']}-w{i}",
                            "opcode": "Drain",
                            "sync_info": {"on_update": [], "on_wait": [wt]},
                        })
                    si["on_wait"] = waits[-keep_n:] if keep_n else []
                    inst["sync_info"] = si
                out.append(inst)
            bb["instructions"] = out
    return json.dumps(m).encode() if changed else raw


def _build():
    nc = bass.Bass("TRN2", target_bir_lowering=False, debug=False, num_devices=1)
    x = nc.dram_tensor("x", [DLOC, 264, 264], F32, kind="ExternalInput").ap()
    dmask = nc.dram_tensor("dmask", [NPART, 1], F32, kind="ExternalInput").ap()
    y = nc.dram_tensor("y", [32, H, W], I8, kind="ExternalOutput").ap()

    _n = [0]

    def _ctr():
        _n[0] += 1
        return _n[0]

    with TileContext(nc) as tc:
        with tc.tile_pool(name="p", bufs=1) as pool:
            dm = pool.tile([NPART, 1], F32, tag="dm", name="dm0")
            nc.gpsimd.dma_start(out=dm[:], in_=dmask[:])
            zrow = pool.tile([NPART, COLS], F32, tag="zr", name="zr0")
            nc.gpsimd.memset(zrow[:], 0.0)

            for t in range(N_WT):
                c0 = WT_OUT * t
                in_w = min(COLS, 264 - c0)

                def T(tag):
                    return pool.tile([NPART, ROWS, COLS], F32, tag=tag, name=f"{tag}_{t}_{_ctr()}")

                v = nc.vector
                xt = T("S1")
                for s in range(3):
                    nc.gpsimd.dma_start(
                        out=xt[s * DLOC:(s + 1) * DLOC, :, 0:in_w],
                        in_=x[:, STRIP_OFF[s]:STRIP_OFF[s] + ROWS, c0:c0 + in_w],
                    )
                # ---- Gaussian [u,1,u] along w, h, d ----
                tw = T("S2")
                v.tensor_tensor(tw[:, :, 1:51], xt[:, :, 0:50], xt[:, :, 2:52], AL.add)
                smw = T("S3")
                v.scalar_tensor_tensor(smw[:, :, 1:51], tw[:, :, 1:51], U,
                                       xt[:, :, 1:51], AL.mult, AL.add)
                th = T("S2")
                v.tensor_tensor(th[:, 1:93, :], smw[:, 0:92, :], smw[:, 2:94, :], AL.add)
                smwh = T("S1")
                v.scalar_tensor_tensor(smwh[:, 1:93, :], th[:, 1:93, :], U,
                                       smw[:, 1:93, :], AL.mult, AL.add)
                # d-shift staging copies (DMA partition realign; compute stays
                # at partition start 0 per ISA 32-alignment rule)
                sp = T("S7")
                nc.gpsimd.dma_start(out=sp[0:119], in_=smwh[1:120])
                sn = T("S8")
                nc.gpsimd.dma_start(out=sn[1:120], in_=smwh[0:119])
                td = T("S2")
                v.tensor_tensor(td[:], sn[:], sp[:], AL.add)
                sm = T("S3")
                v.scalar_tensor_tensor(sm[:], td[:], U, smwh[:], AL.mult, AL.add)
                # ---- Sobel d-stage: A = sm*[1,1,1]_d, B = sm*[-1,0,1]_d ----
                p2 = T("S7")
                nc.gpsimd.dma_start(out=p2[0:119], in_=sm[1:120])
                m2 = T("S8")
                nc.gpsimd.dma_start(out=m2[1:120], in_=sm[0:119])
                a1 = T("S2")
                v.tensor_tensor(a1[:], p2[:], m2[:], AL.add)
                A = T("S1")
                v.tensor_tensor(A[:], a1[:], sm[:], AL.add)
                B = T("S2")
                v.tensor_tensor(B[:], p2[:], m2[:], AL.subtract)
                # ---- gx = A *h [1,2,1] *w [-1,0,1] ----
                ph = T("S3")
                v.tensor_tensor(ph[:, 2:92, :], A[:, 1:91, :], A[:, 3:93, :], AL.add)
                gxh = T("S4")
                v.scalar_tensor_tensor(gxh[:, 2:92, :], A[:, 2:92, :], 2.0,
                                       ph[:, 2:92, :], AL.mult, AL.add)
                gx = T("S3")
                v.tensor_tensor(gx[:, :, 2:50], gxh[:, :, 3:51], gxh[:, :, 1:49],
                                AL.subtract)
                # ---- gy = A *h [-1,0,1] *w [1,2,1] ----
                gyh = T("S4")
                v.tensor_tensor(gyh[:, 2:92, :], A[:, 3:93, :], A[:, 1:91, :],
                                AL.subtract)
                pw = T("S5")
                v.tensor_tensor(pw[:, :, 2:50], gyh[:, :, 1:49], gyh[:, :, 3:51], AL.add)
                gy = T("S6")
                v.scalar_tensor_tensor(gy[:, :, 2:50], gyh[:, :, 2:50], 2.0,
                                       pw[:, :, 2:50], AL.mult, AL.add)
                # ---- gz = B *h [1,1,1] *w [1,1,1] ----
                bh1 = T("S1")
                v.tensor_tensor(bh1[:, 2:92, :], B[:, 1:91, :], B[:, 3:93, :], AL.add)
                bh = T("S4")
                v.tensor_tensor(bh[:, 2:92, :], bh1[:, 2:92, :], B[:, 2:92, :], AL.add)
                bw1 = T("S1")
                v.tensor_tensor(bw1[:, :, 2:50], bh[:, :, 1:49], bh[:, :, 3:51], AL.add)
                gz = T("S2")
                v.tensor_tensor(gz[:, :, 2:50], bw1[:, :, 2:50], bh[:, :, 2:50], AL.add)
                # ---- msq = dmask*(gx^2+gy^2+gz^2), then h/w border zeroing ----
                sx = T("S1")
                nc.scalar.activation(sx[:], gx[:], SQ, scale=dm[:, 0:1])
                sy = T("S4")
                nc.scalar.activation(sy[:], gy[:], SQ, scale=dm[:, 0:1])
                sz = T("S6")
                nc.scalar.activation(sz[:], gz[:], SQ, scale=dm[:, 0:1])
                m1 = T("S2")
                v.tensor_tensor(m1[:], sx[:], sy[:], AL.add)
                msq = T("S1")
                v.tensor_tensor(msq[:], m1[:], sz[:], AL.add)
                nc.gpsimd.dma_start(out=msq[0:40, 4:5, :], in_=zrow[0:40, :])
                nc.gpsimd.dma_start(out=msq[80:120, 89:90, :], in_=zrow[80:120, :])
                if t == 0:
                    nc.gpsimd.memset(msq[:, :, 4:5], 0.0)
                if t == N_WT - 1:
                    nc.gpsimd.memset(msq[:, :, 39:40], 0.0)
                # ---- NMS ----
                r2 = T("S2")
                v.tensor_tensor(r2[:, :, 3:49], msq[:, :, 2:48], msq[:, :, 4:50], AL.max)
                r3 = T("S3")
                v.tensor_tensor(r3[:, :, 3:49], r2[:, :, 3:49], msq[:, :, 3:49], AL.max)
                mh = T("S4")
                v.tensor_tensor(mh[:, 3:91, :], r3[:, 2:90, :], r3[:, 4:92, :], AL.max)
                nb8 = T("S3")
                v.tensor_tensor(nb8[:, 3:91, :], mh[:, 3:91, :], r2[:, 3:91, :], AL.max)
                nbm = T("S7")
                nc.gpsimd.dma_start(out=nbm[1:120], in_=nb8[0:119])
                keep = T("S2")
                v.tensor_tensor(keep[:], msq[:], nbm[:], AL.is_gt)
                nmsq = T("S3")
                v.tensor_tensor(nmsq[:], msq[:], keep[:], AL.mult)
                # ---- thresholds ----
                strong = T("S1")
                v.tensor_scalar(strong[:], nmsq[:], HI2, None, AL.is_gt)
                weakish = T("S2")
                v.tensor_scalar(weakish[:], nmsq[:], LO2, None, AL.is_gt)
                weak = T("S3")
                v.tensor_tensor(weak[:], weakish[:], strong[:], AL.subtract)
                # ---- hysteresis ----
                tp = T("S7")
                nc.gpsimd.dma_start(out=tp[0:119], in_=strong[1:120])
                tm = T("S8")
                nc.gpsimd.dma_start(out=tm[1:120], in_=strong[0:119])
                sd = T("S2")
                v.tensor_tensor(sd[:], tp[:], tm[:], AL.add)
                sh = T("S4")
                v.tensor_tensor(sh[:, 4:90, :], strong[:, 3:89, :], strong[:, 5:91, :],
                                AL.add)
                sw = T("S5")
                v.tensor_tensor(sw[:, :, 4:48], strong[:, :, 3:47], strong[:, :, 5:49],
                                AL.add)
                sa = T("S6")
                v.tensor_tensor(sa[:], sd[:], sh[:], AL.add)
                any6 = T("S2")
                v.tensor_tensor(any6[:], sa[:], sw[:], AL.add)
                wa = T("S4")
                v.scalar_tensor_tensor(wa[:], any6[:], 0.5, weak[:], AL.is_ge, AL.mult)
                out01 = pool.tile([NPART, ROWS, COLS], I8, tag="o8", name=f"o8_{t}")
                v.tensor_tensor(out01[:], wa[:], strong[:], AL.max)

                ow = WT_OUT if t < N_WT - 1 else 36
                for s in range(3):
                    r0, nr, h0 = STRIP_OUT[s]
                    nc.gpsimd.dma_start(
                        out=y[:, h0:h0 + nr, WT_OUT * t:WT_OUT * t + ow],
                        in_=out01[s * DLOC + 4:s * DLOC + 36, r0:r0 + nr, 4:4 + ow],
                    )
    orig = nc.to_json_bytes
    nc.to_json_bytes = lambda: _fix_bir_json_bytes(orig())
    return nc


_NC_CACHE = None


def kernel(x: np.ndarray) -> np.ndarray:
    global _NC_CACHE
    x3 = np.ascontiguousarray(x[0], dtype=np.float32)
    xp = np.pad(x3, 1, mode="reflect")                # (258,258,258)
    xp = np.pad(xp, ((0, 0), (3, 3), (3, 3)))         # (258,264,264)

    in_maps = []
    for c in range(N_CORES):
        g0 = 32 * c
        slab = np.zeros((DLOC, 264, 264), np.float32)
        lo = max(0, g0 - 3)            # xp d-index = global+1, want [g0-3, g0+37)
        hi = min(258, g0 + 37)
        slab[lo - (g0 - 3):hi - (g0 - 3)] = xp[lo:hi]
        dmv = np.ones((NPART, 1), np.float32)
        if c == 0:
            dmv[[4, 44, 84]] = 0.0
        if c == N_CORES - 1:
            dmv[[35, 75, 115]] = 0.0
        in_maps.append({"x": slab, "dmask": dmv})

    if _NC_CACHE is None:
        _NC_CACHE = _build()
    res = run_bass_kernel_spmd(_NC_CACHE, in_maps, list(range(N_CORES)))
    out = np.concatenate([r["y"] for r in res.results], axis=0)
    return out[None].astype(np.int8)



# revision 2
# speedup vs baseline: 1.9558x; 1.9558x over previous
"""3D Canny edge detector on 8 Trainium2 cores.

Shard D=256 across 8 cores (32 output slices each) with a 4-voxel halo,
entirely host-side (no collectives). Per-core layout: partitions =
3 h-strips x 40 local d-slices (120 of 128), free dim = (94 h-rows, 48 w-cols)
per w-tile. All three stencil axes are then partition- or free-dim shifts.

Wall time here is dominated by the axon host<->device tunnel (~45MB/s), so
the input is quantized host-side to int16 (uint16 grid XOR 0x8000; the
-32768 offset cancels exactly in the Sobel gradients) and the binary output
is bit-packed on device to uint8 (8 voxels/byte, w-major little-endian),
then unpacked host-side with np.unpackbits. sqrt is eliminated by comparing
squared magnitudes against squared thresholds; the Gaussian is applied
unnormalized ([u,1,u] per axis) with normalization and the 65535 input
scale folded into the thresholds. Global-border zeroing of the gradient
magnitude is done via a per-partition mask input (d borders, differs per
core) fused into the ScalarE square ops, plus tiny memsets for the h/w
border rows/cols.
"""
import json
import numpy as np

import concourse.bass as bass
import concourse.mybir as mybir
from concourse.bass_utils import run_bass_kernel_spmd
from concourse.tile import TileContext

F32 = mybir.dt.float32
I16 = mybir.dt.int16
U8 = mybir.dt.uint8
AL = mybir.AluOpType
SQ = mybir.ActivationFunctionType.Square

N_CORES = 8
D, H, W = 256, 256, 256
DLOC = 40           # 32 output slices + 4 halo each side
NPART = 120         # 3 strips * 40
ROWS = 94           # h rows per strip tile (out rows + up to 4 halo each side)
COLS = 48           # w cols per tile (40 out + 4 halo each side)
WT_OUT = 40
N_WT = 7
STRIP_OFF = (0, 85, 170)                       # padded-h offset per strip
STRIP_OUT = ((4, 86, 0), (5, 85, 86), (5, 85, 171))  # (first r, n rows, h0)

U = float(np.exp(np.float64(-0.5)))
SC = (1.0 + 2.0 * U) ** 3
QF = 65535.0
HI2 = float((0.2 * SC * QF) ** 2)
LO2 = float((0.1 * SC * QF) ** 2)


def _fix_bir_json_bytes(raw: bytes) -> bytes:
    """walrus codegen has per-instruction sync-wait-slot limits (1 for CTRL
    Drain, 2 for compute structs). Hoist excess waits onto prepended
    single-wait Drain instructions on the same engine."""
    m = json.loads(raw)
    changed = False
    for fn in m.get("functions", []):
        for bb in fn.get("blocks", []):
            out = []
            for inst in bb.get("instructions", []):
                si = inst.get("sync_info") or {}
                waits = si.get("on_wait") or []
                lim = 1
                if len(waits) > lim and inst.get("engine") not in (None, "Unassigned"):
                    changed = True
                    keep_n = lim
                    for i, wt in enumerate(waits[:-keep_n] if keep_n else waits):
                        out.append({
                            "debug": inst.get("debug", 0),
                            "engine": inst["engine"],
                            "ins": [], "outs": [],
                            "is_reset_sema": False,
                            "name": f"{inst['name']}-w{i}",
                            "opcode": "Drain",
                            "sync_info": {"on_update": [], "on_wait": [wt]},
                        })
                    si["on_wait"] = waits[-keep_n:] if keep_n else []
                    inst["sync_info"] = si
                out.append(inst)
            bb["instructions"] = out
    return json.dumps(m).encode() if changed else raw


def _build():
    nc = bass.Bass("TRN2", target_bir_lowering=False, debug=False, num_devices=1)
    x = nc.dram_tensor("x", [DLOC, 264, 264], I16, kind="ExternalInput").ap()
    dmask = nc.dram_tensor("dmask", [NPART, 1], F32, kind="ExternalInput").ap()
    y = nc.dram_tensor("y", [32, H, 32], U8, kind="ExternalOutput").ap()

    _n = [0]

    def _ctr():
        _n[0] += 1
        return _n[0]

    with TileContext(nc) as tc:
        with tc.tile_pool(name="p", bufs=1) as pool:
            dm = pool.tile([NPART, 1], F32, tag="dm", name="dm0")
            nc.gpsimd.dma_start(out=dm[:], in_=dmask[:])
            zrow = pool.tile([NPART, COLS], F32, tag="zr", name="zr0")
            nc.gpsimd.memset(zrow[:], 0.0)

            for t in range(N_WT):
                c0 = WT_OUT * t
                in_w = min(COLS, 264 - c0)

                def T(tag, cols=COLS, dt=F32):
                    return pool.tile([NPART, ROWS, cols], dt, tag=tag,
                                     name=f"{tag}_{t}_{_ctr()}")

                v = nc.vector
                xu = T("S9", dt=I16)
                for s in range(3):
                    nc.gpsimd.dma_start(
                        out=xu[s * DLOC:(s + 1) * DLOC, :, 0:in_w],
                        in_=x[:, STRIP_OFF[s]:STRIP_OFF[s] + ROWS, c0:c0 + in_w],
                    )
                xt = T("S1")
                nc.scalar.copy(xt[:], xu[:])
                # ---- Gaussian [u,1,u] along w, h, d ----
                tw = T("S2")
                v.tensor_tensor(tw[:, :, 1:47], xt[:, :, 0:46], xt[:, :, 2:48], AL.add)
                smw = T("S3")
                v.scalar_tensor_tensor(smw[:, :, 1:47], tw[:, :, 1:47], U,
                                       xt[:, :, 1:47], AL.mult, AL.add)
                th = T("S2")
                v.tensor_tensor(th[:, 1:93, :], smw[:, 0:92, :], smw[:, 2:94, :], AL.add)
                smwh = T("S1")
                v.scalar_tensor_tensor(smwh[:, 1:93, :], th[:, 1:93, :], U,
                                       smw[:, 1:93, :], AL.mult, AL.add)
                # d-shift staging copies (DMA partition realign; compute stays
                # at partition start 0 per ISA 32-alignment rule)
                sp = T("S7")
                nc.gpsimd.dma_start(out=sp[0:119], in_=smwh[1:120])
                sn = T("S8")
                nc.gpsimd.dma_start(out=sn[1:120], in_=smwh[0:119])
                td = T("S2")
                v.tensor_tensor(td[:], sn[:], sp[:], AL.add)
                sm = T("S3")
                v.scalar_tensor_tensor(sm[:], td[:], U, smwh[:], AL.mult, AL.add)
                # ---- Sobel d-stage: A = sm*[1,1,1]_d, B = sm*[-1,0,1]_d ----
                p2 = T("S7")
                nc.gpsimd.dma_start(out=p2[0:119], in_=sm[1:120])
                m2 = T("S8")
                nc.gpsimd.dma_start(out=m2[1:120], in_=sm[0:119])
                a1 = T("S2")
                v.tensor_tensor(a1[:], p2[:], m2[:], AL.add)
                A = T("S1")
                v.tensor_tensor(A[:], a1[:], sm[:], AL.add)
                B = T("S2")
                v.tensor_tensor(B[:], p2[:], m2[:], AL.subtract)
                # ---- gx = A *h [1,2,1] *w [-1,0,1] ----
                ph = T("S3")
                v.tensor_tensor(ph[:, 2:92, :], A[:, 1:91, :], A[:, 3:93, :], AL.add)
                gxh = T("S4")
                v.scalar_tensor_tensor(gxh[:, 2:92, :], A[:, 2:92, :], 2.0,
                                       ph[:, 2:92, :], AL.mult, AL.add)
                gx = T("S3")
                v.tensor_tensor(gx[:, :, 2:46], gxh[:, :, 3:47], gxh[:, :, 1:45],
                                AL.subtract)
                # ---- gy = A *h [-1,0,1] *w [1,2,1] ----
                gyh = T("S4")
                v.tensor_tensor(gyh[:, 2:92, :], A[:, 3:93, :], A[:, 1:91, :],
                                AL.subtract)
                pw = T("S5")
                v.tensor_tensor(pw[:, :, 2:46], gyh[:, :, 1:45], gyh[:, :, 3:47], AL.add)
                gy = T("S6")
                v.scalar_tensor_tensor(gy[:, :, 2:46], gyh[:, :, 2:46], 2.0,
                                       pw[:, :, 2:46], AL.mult, AL.add)
                # ---- gz = B *h [1,1,1] *w [1,1,1] ----
                bh1 = T("S1")
                v.tensor_tensor(bh1[:, 2:92, :], B[:, 1:91, :], B[:, 3:93, :], AL.add)
                bh = T("S4")
                v.tensor_tensor(bh[:, 2:92, :], bh1[:, 2:92, :], B[:, 2:92, :], AL.add)
                bw1 = T("S1")
                v.tensor_tensor(bw1[:, :, 2:46], bh[:, :, 1:45], bh[:, :, 3:47], AL.add)
                gz = T("S2")
                v.tensor_tensor(gz[:, :, 2:46], bw1[:, :, 2:46], bh[:, :, 2:46], AL.add)
                # ---- msq = dmask*(gx^2+gy^2+gz^2), then h/w border zeroing ----
                sx = T("S1")
                nc.scalar.activation(sx[:], gx[:], SQ, scale=dm[:, 0:1])
                sy = T("S4")
                nc.scalar.activation(sy[:], gy[:], SQ, scale=dm[:, 0:1])
                sz = T("S6")
                nc.scalar.activation(sz[:], gz[:], SQ, scale=dm[:, 0:1])
                m1 = T("S2")
                v.tensor_tensor(m1[:], sx[:], sy[:], AL.add)
                msq = T("S1")
                v.tensor_tensor(msq[:], m1[:], sz[:], AL.add)
                nc.gpsimd.dma_start(out=msq[0:40, 4:5, :], in_=zrow[0:40, :])
                nc.gpsimd.dma_start(out=msq[80:120, 89:90, :], in_=zrow[80:120, :])
                if t == 0:
                    nc.gpsimd.memset(msq[:, :, 4:5], 0.0)
                if t == N_WT - 1:
                    nc.gpsimd.memset(msq[:, :, 19:20], 0.0)
                # ---- NMS ----
                r2 = T("S2")
                v.tensor_tensor(r2[:, :, 3:45], msq[:, :, 2:44], msq[:, :, 4:46], AL.max)
                r3 = T("S3")
                v.tensor_tensor(r3[:, :, 3:45], r2[:, :, 3:45], msq[:, :, 3:45], AL.max)
                mh = T("S4")
                v.tensor_tensor(mh[:, 3:91, :], r3[:, 2:90, :], r3[:, 4:92, :], AL.max)
                nb8 = T("S3")
                v.tensor_tensor(nb8[:, 3:91, :], mh[:, 3:91, :], r2[:, 3:91, :], AL.max)
                nbm = T("S7")
                nc.gpsimd.dma_start(out=nbm[1:120], in_=nb8[0:119])
                keep = T("S2")
                v.tensor_tensor(keep[:], msq[:], nbm[:], AL.is_gt)
                nmsq = T("S3")
                v.tensor_tensor(nmsq[:], msq[:], keep[:], AL.mult)
                # ---- thresholds ----
                strong = T("S1")
                v.tensor_scalar(strong[:], nmsq[:], HI2, None, AL.is_gt)
                weakish = T("S2")
                v.tensor_scalar(weakish[:], nmsq[:], LO2, None, AL.is_gt)
                weak = T("S3")
                v.tensor_tensor(weak[:], weakish[:], strong[:], AL.subtract)
                # ---- hysteresis ----
                tp = T("S7")
                nc.gpsimd.dma_start(out=tp[0:119], in_=strong[1:120])
                tm = T("S8")
                nc.gpsimd.dma_start(out=tm[1:120], in_=strong[0:119])
                sd = T("S2")
                v.tensor_tensor(sd[:], tp[:], tm[:], AL.add)
                sh = T("S4")
                v.tensor_tensor(sh[:, 4:90, :], strong[:, 3:89, :], strong[:, 5:91, :],
                                AL.add)
                sw = T("S5")
                v.tensor_tensor(sw[:, :, 4:44], strong[:, :, 3:43], strong[:, :, 5:45],
                                AL.add)
                sa = T("S6")
                v.tensor_tensor(sa[:], sd[:], sh[:], AL.add)
                any6 = T("S2")
                v.tensor_tensor(any6[:], sa[:], sw[:], AL.add)
                wa = T("S4")
                v.scalar_tensor_tensor(wa[:], any6[:], 0.5, weak[:], AL.is_ge, AL.mult)
                of = T("S6")
                v.tensor_tensor(of[:], wa[:], strong[:], AL.max)
                # ---- bit-pack 8 w-voxels/byte (little-endian) ----
                pk1 = T("K1", cols=20)
                v.scalar_tensor_tensor(pk1[:, :, 0:20], of[:, :, 5:45:2], 2.0,
                                       of[:, :, 4:44:2], AL.mult, AL.add)
                pk2 = T("K2", cols=10)
                v.scalar_tensor_tensor(pk2[:, :, 0:10], pk1[:, :, 1:20:2], 4.0,
                                       pk1[:, :, 0:20:2], AL.mult, AL.add)
                pk3 = T("K3", cols=5, dt=U8)
                v.scalar_tensor_tensor(pk3[:, :, 0:5], pk2[:, :, 1:10:2], 16.0,
                                       pk2[:, :, 0:10:2], AL.mult, AL.add)

                ob = 5 if t < N_WT - 1 else 2
                for s in range(3):
                    r0, nr, h0 = STRIP_OUT[s]
                    nc.gpsimd.dma_start(
                        out=y[:, h0:h0 + nr, 5 * t:5 * t + ob],
                        in_=pk3[s * DLOC + 4:s * DLOC + 36, r0:r0 + nr, 0:ob],
                    )
    orig = nc.to_json_bytes
    nc.to_json_bytes = lambda: _fix_bir_json_bytes(orig())
    return nc


_NC_CACHE = None


def kernel(x: np.ndarray) -> np.ndarray:
    global _NC_CACHE
    x3 = np.asarray(x[0], dtype=np.float32)
    q = (x3 * np.float32(QF) + np.float32(0.5)).astype(np.uint16)
    s16 = (q ^ np.uint16(0x8000)).view(np.int16)
    xp = np.pad(s16, 1, mode="reflect")               # (258,258,258)
    xp = np.pad(xp, ((0, 0), (3, 3), (3, 3)))         # (258,264,264)

    in_maps = []
    for c in range(N_CORES):
        g0 = 32 * c
        slab = np.zeros((DLOC, 264, 264), np.int16)
        lo = max(0, g0 - 3)            # xp d-index = global+1, want [g0-3, g0+37)
        hi = min(258, g0 + 37)
        slab[lo - (g0 - 3):hi - (g0 - 3)] = xp[lo:hi]
        dmv = np.ones((NPART, 1), np.float32)
        if c == 0:
            dmv[[4, 44, 84]] = 0.0
        if c == N_CORES - 1:
            dmv[[35, 75, 115]] = 0.0
        in_maps.append({"x": slab, "dmask": dmv})

    if _NC_CACHE is None:
        _NC_CACHE = _build()
    res = run_bass_kernel_spmd(_NC_CACHE, in_maps, list(range(N_CORES)))
    yp = np.concatenate([r["y"] for r in res.results], axis=0)  # (256,256,32) u8
    bits = np.unpackbits(yp, axis=-1, bitorder="little")        # (256,256,256)
    return bits[None].astype(np.int8)


# revision 7
# speedup vs baseline: 2.2563x; 1.1537x over previous
"""3D Canny edge detector on 8 Trainium2 cores.

Shard D=256 across 8 cores (32 slices each), *disjoint* host-side transfer
(the axon host<->device tunnel at ~45MB/s dominates wall time), with the
4-slice d-halo exchanged on device via an AllGather of each core's 8
boundary slices; neighbor blocks are selected with partition_id-derived
dynamic DMA offsets. A padded (40,264,264) slab is then assembled in
device DRAM: disjoint block + halos + h/w reflect fix-ups bounced through
SBUF (the 3-voxel zero margins are left uninitialized - only their
finiteness matters, every consumer is masked). The d-direction reflect at
the global d=0/255 borders cannot come from a neighbor, so it is applied
as a per-partition linear correction (dsel input, +-u at the border
partitions) to the smoothed volume: sm += dsel * (sp - sn) replaces the
missing/garbage smwh[d-1] term with smwh[d+1] exactly.

Input is quantized host-side to int16 (uint16 grid XOR 0x8000; the -32768
offset cancels exactly in the zero-sum Sobel kernels) and the binary
output is bit-packed on device to uint8 (8 voxels/byte along w,
little-endian), unpacked host-side with np.unpackbits. sqrt is eliminated
by comparing squared magnitudes against squared thresholds; the Gaussian
is applied unnormalized ([u,1,u] per axis) with normalization and the
65535 input scale folded into the thresholds. Per-core layout: partitions
= 3 h-strips x 40 local d-slices, free dim = (94 h-rows, 48 w-cols) per
w-tile; all three stencil axes are partition- or free-dim shifts.
"""
import json
import numpy as np

import concourse.bass as bass
import concourse.mybir as mybir
from concourse.bass_utils import run_bass_kernel_spmd
from concourse.tile import TileContext

F32 = mybir.dt.float32
I16 = mybir.dt.int16
U8 = mybir.dt.uint8
AL = mybir.AluOpType
SQ = mybir.ActivationFunctionType.Square
CP = mybir.ActivationFunctionType.Copy

N_CORES = 8
D, H, W = 256, 256, 256
DSH = 32            # disjoint d slices per core
DLOC = 40           # 32 output slices + 4 halo each side
NPART = 120         # 3 strips * 40
ROWS = 94           # h rows per strip tile (out rows + up to 4 halo each side)
COLS = 48           # w cols per tile (40 out + 4 halo each side)
WT_OUT = 40
N_WT = 7
STRIP_OFF = (0, 85, 170)                       # padded-h offset per strip
STRIP_OUT = ((4, 86, 0), (5, 85, 86), (5, 85, 171))  # (first r, n rows, h0)

U = float(np.exp(np.float64(-0.5)))
SC = (1.0 + 2.0 * U) ** 3
QF = 65535.0
HI2 = float((0.2 * SC * QF) ** 2)
LO2 = float((0.1 * SC * QF) ** 2)


def _fix_bir_json_bytes(raw: bytes) -> bytes:
    """walrus codegen has per-instruction sync-wait-slot limits (1 for CTRL
    Drain, 2 for compute structs). Hoist excess waits onto prepended
    single-wait Drain instructions on the same engine."""
    m = json.loads(raw)
    changed = False
    for fn in m.get("functions", []):
        for bb in fn.get("blocks", []):
            out = []
            for inst in bb.get("instructions", []):
                si = inst.get("sync_info") or {}
                waits = si.get("on_wait") or []
                lim = 1
                if len(waits) > lim and inst.get("engine") not in (None, "Unassigned"):
                    changed = True
                    keep_n = lim
                    for i, wt in enumerate(waits[:-keep_n] if keep_n else waits):
                        out.append({
                            "debug": inst.get("debug", 0),
                            "engine": inst["engine"],
                            "ins": [], "outs": [],
                            "is_reset_sema": False,
                            "name": f"{inst['name']}-w{i}",
                            "opcode": "Drain",
                            "sync_info": {"on_update": [], "on_wait": [wt]},
                        })
                    si["on_wait"] = waits[-keep_n:] if keep_n else []
                    inst["sync_info"] = si
                out.append(inst)
            bb["instructions"] = out
    return json.dumps(m).encode() if changed else raw


def _build():
    nc = bass.Bass("TRN2", target_bir_lowering=False, debug=False, num_devices=8)
    x = nc.dram_tensor("x", [DSH, 256, 256], I16, kind="ExternalInput").ap()
    dmask = nc.dram_tensor("dmask", [NPART, 1], F32, kind="ExternalInput").ap()
    dsel = nc.dram_tensor("dsel", [NPART, 1], F32, kind="ExternalInput").ap()
    y = nc.dram_tensor("y", [32, H, 32], U8, kind="ExternalOutput").ap()
    bnd = nc.dram_tensor("bnd", [8, 256, 256], I16, kind="Internal").ap()
    agb = nc.dram_tensor("agb", [64, 256, 256], I16, kind="Internal").ap()
    slab = nc.dram_tensor("slab", [DLOC, 264, 264], I16, kind="Internal").ap()

    _n = [0]

    def _ctr():
        _n[0] += 1
        return _n[0]

    with TileContext(nc) as tc:
        with tc.tile_pool(name="p", bufs=1) as pool:
            dm = pool.tile([NPART, 1], F32, tag="dm", name="dm0")
            nc.gpsimd.dma_start(out=dm[:], in_=dmask[:])
            dsl = pool.tile([NPART, 1], F32, tag="dsl", name="dsl0")
            nc.gpsimd.dma_start(out=dsl[:], in_=dsel[:])
            zrow = pool.tile([NPART, COLS], F32, tag="zr", name="zr0")
            nc.gpsimd.memset(zrow[:], 0.0)

            # ---- halo exchange: own boundary slices -> AllGather ----
            nc.gpsimd.dma_start(out=bnd[0:4], in_=x[0:4])
            nc.gpsimd.dma_start(out=bnd[4:8], in_=x[DSH - 4:DSH])
            nc.gpsimd.collective_compute(
                "AllGather", mybir.AluOpType.bypass,
                replica_groups=[list(range(N_CORES))],
                ins=[bnd[:].opt()], outs=[agb[:].opt()],
            )
            pid = nc.gpsimd.partition_id()
            top_off = ((pid + (N_CORES - 1)) % N_CORES) * 8 + 4
            bot_off = ((pid + 1) % N_CORES) * 8
            # ---- assemble padded slab in DRAM ----
            nc.gpsimd.dma_start(out=slab[0:4, 4:260, 4:260],
                                in_=agb[bass.ds(top_off, 4)])
            nc.gpsimd.dma_start(out=slab[4:36, 4:260, 4:260], in_=x[:])
            nc.gpsimd.dma_start(out=slab[36:40, 4:260, 4:260],
                                in_=agb[bass.ds(bot_off, 4)])
            # h/w reflect fix-ups are applied in SBUF on the loaded tiles
            # below (staged like the d-shift copies); slab's h/w margin
            # rows/cols 0:4 and 260:264 stay uninitialized - only their
            # finiteness matters, every consumer is masked or overwritten.

            for t in range(N_WT):
                c0 = WT_OUT * t
                in_w = min(COLS, 264 - c0)

                def T(tag, cols=COLS, dt=F32):
                    return pool.tile([NPART, ROWS, cols], dt, tag=tag,
                                     name=f"{tag}_{t}_{_ctr()}")

                v = nc.vector
                xu = T("S9", dt=I16)
                for s in range(3):
                    nc.gpsimd.dma_start(
                        out=xu[s * DLOC:(s + 1) * DLOC, :, 0:in_w],
                        in_=slab[:, STRIP_OFF[s]:STRIP_OFF[s] + ROWS, c0:c0 + in_w],
                    )
                # h reflect: strip 0 padded row 3 <- raw row 1 (tile row 5);
                # strip 2 padded row 260 (tile row 90) <- raw 254 (row 88).
                rf = pool.tile([NPART, 1, COLS], I16, tag="RF",
                               name=f"rf_{t}")
                nc.gpsimd.dma_start(out=rf[0:40, 0:1, :], in_=xu[0:40, 5:6, :])
                nc.gpsimd.dma_start(out=xu[0:40, 3:4, :], in_=rf[0:40, 0:1, :])
                nc.gpsimd.dma_start(out=rf[80:120, 0:1, :], in_=xu[80:120, 88:89, :])
                nc.gpsimd.dma_start(out=xu[80:120, 90:91, :], in_=rf[80:120, 0:1, :])
                # w reflect (after rows, so corners inherit the row fix):
                # t=0 padded col 3 <- raw col 1 (col 5); t=6 padded col 260
                # (col 20) <- raw col 254 (col 18).
                if t == 0 or t == N_WT - 1:
                    csrc, cdst = (5, 3) if t == 0 else (18, 20)
                    cf = pool.tile([NPART, ROWS, 1], I16, tag="CF",
                                   name=f"cf_{t}")
                    nc.gpsimd.dma_start(out=cf[:], in_=xu[:, :, csrc:csrc + 1])
                    nc.gpsimd.dma_start(out=xu[:, :, cdst:cdst + 1], in_=cf[:])
                xt = T("S1")
                nc.scalar.copy(xt[:], xu[:])
                # ---- Gaussian [u,1,u] along w, h, d ----
                tw = T("S2")
                v.tensor_tensor(tw[:, :, 1:47], xt[:, :, 0:46], xt[:, :, 2:48], AL.add)
                smw = T("S3")
                v.scalar_tensor_tensor(smw[:, :, 1:47], tw[:, :, 1:47], U,
                                       xt[:, :, 1:47], AL.mult, AL.add)
                th = T("S2")
                v.tensor_tensor(th[:, 1:93, :], smw[:, 0:92, :], smw[:, 2:94, :], AL.add)
                smwh = T("S1")
                v.scalar_tensor_tensor(smwh[:, 1:93, :], th[:, 1:93, :], U,
                                       smw[:, 1:93, :], AL.mult, AL.add)
                # d-shift staging copies (DMA partition realign; compute stays
                # at partition start 0 per ISA 32-alignment rule)
                sp = T("S7")
                nc.gpsimd.dma_start(out=sp[0:119], in_=smwh[1:120])
                sn = T("S8")
                nc.gpsimd.dma_start(out=sn[1:120], in_=smwh[0:119])
                td = T("S2")
                v.tensor_tensor(td[:], sn[:], sp[:], AL.add)
                sm0 = T("S3")
                v.scalar_tensor_tensor(sm0[:], td[:], U, smwh[:], AL.mult, AL.add)
                # d-reflect correction at global d borders: sm += dsel*(sp-sn)
                diff = T("S2")
                v.tensor_tensor(diff[:], sp[:], sn[:], AL.subtract)
                fixt = T("S1")
                nc.scalar.activation(fixt[:], diff[:], CP, scale=dsl[:, 0:1])
                sm = T("S2")
                v.tensor_tensor(sm[:], sm0[:], fixt[:], AL.add)
                # ---- Sobel d-stage: A = sm*[1,1,1]_d, B = sm*[-1,0,1]_d ----
                p2 = T("S7")
                nc.gpsimd.dma_start(out=p2[0:119], in_=sm[1:120])
                m2 = T("S8")
                nc.gpsimd.dma_start(out=m2[1:120], in_=sm[0:119])
                a1 = T("S1")
                v.tensor_tensor(a1[:], p2[:], m2[:], AL.add)
                A = T("S3")
                v.tensor_tensor(A[:], a1[:], sm[:], AL.add)
                B = T("S2")
                v.tensor_tensor(B[:], p2[:], m2[:], AL.subtract)
                # ---- gx = A *h [1,2,1] *w [-1,0,1] ----
                ph = T("S1")
                v.tensor_tensor(ph[:, 2:92, :], A[:, 1:91, :], A[:, 3:93, :], AL.add)
                gxh = T("S4")
                v.scalar_tensor_tensor(gxh[:, 2:92, :], A[:, 2:92, :], 2.0,
                                       ph[:, 2:92, :], AL.mult, AL.add)
                gx = T("S1")
                v.tensor_tensor(gx[:, :, 2:46], gxh[:, :, 3:47], gxh[:, :, 1:45],
                                AL.subtract)
                # ---- gy = A *h [-1,0,1] *w [1,2,1] ----
                gyh = T("S5")
                v.tensor_tensor(gyh[:, 2:92, :], A[:, 3:93, :], A[:, 1:91, :],
                                AL.subtract)
                pw = T("S6")
                v.tensor_tensor(pw[:, :, 2:46], gyh[:, :, 1:45], gyh[:, :, 3:47], AL.add)
                gy = T("S4")
                v.scalar_tensor_tensor(gy[:, :, 2:46], gyh[:, :, 2:46], 2.0,
                                       pw[:, :, 2:46], AL.mult, AL.add)
                # ---- gz = B *h [1,1,1] *w [1,1,1] ----
                bh1 = T("S7")
                v.tensor_tensor(bh1[:, 2:92, :], B[:, 1:91, :], B[:, 3:93, :], AL.add)
                bh = T("S8")
                v.tensor_tensor(bh[:, 2:92, :], bh1[:, 2:92, :], B[:, 2:92, :], AL.add)
                bw1 = T("S5")
                v.tensor_tensor(bw1[:, :, 2:46], bh[:, :, 1:45], bh[:, :, 3:47], AL.add)
                gz = T("S2")
                v.tensor_tensor(gz[:, :, 2:46], bw1[:, :, 2:46], bh[:, :, 2:46], AL.add)
                # ---- msq = dmask*(gx^2+gy^2+gz^2), then h/w border zeroing ----
                sx = T("S7")
                nc.scalar.activation(sx[:], gx[:], SQ, scale=dm[:, 0:1])
                sy = T("S1")
                nc.scalar.activation(sy[:], gy[:], SQ, scale=dm[:, 0:1])
                sz = T("S5")
                nc.scalar.activation(sz[:], gz[:], SQ, scale=dm[:, 0:1])
                m1 = T("S2")
                v.tensor_tensor(m1[:], sx[:], sy[:], AL.add)
                msq = T("S4")
                v.tensor_tensor(msq[:], m1[:], sz[:], AL.add)
                nc.gpsimd.dma_start(out=msq[0:40, 4:5, :], in_=zrow[0:40, :])
                nc.gpsimd.dma_start(out=msq[80:120, 89:90, :], in_=zrow[80:120, :])
                if t == 0:
                    nc.gpsimd.memset(msq[:, :, 4:5], 0.0)
                if t == N_WT - 1:
                    nc.gpsimd.memset(msq[:, :, 19:20], 0.0)
                # ---- NMS ----
                r2 = T("S1")
                v.tensor_tensor(r2[:, :, 3:45], msq[:, :, 2:44], msq[:, :, 4:46], AL.max)
                r3 = T("S5")
                v.tensor_tensor(r3[:, :, 3:45], r2[:, :, 3:45], msq[:, :, 3:45], AL.max)
                mh = T("S2")
                v.tensor_tensor(mh[:, 3:91, :], r3[:, 2:90, :], r3[:, 4:92, :], AL.max)
                nb8 = T("S5")
                v.tensor_tensor(nb8[:, 3:91, :], mh[:, 3:91, :], r2[:, 3:91, :], AL.max)
                nbm = T("S7")
                nc.gpsimd.dma_start(out=nbm[1:120], in_=nb8[0:119])
                keep = T("S1")
                v.tensor_tensor(keep[:], msq[:], nbm[:], AL.is_gt)
                nmsq = T("S2")
                v.tensor_tensor(nmsq[:], msq[:], keep[:], AL.mult)
                # ---- thresholds ----
                strong = T("S4")
                v.tensor_scalar(strong[:], nmsq[:], HI2, None, AL.is_gt)
                weakish = T("S1")
                v.tensor_scalar(weakish[:], nmsq[:], LO2, None, AL.is_gt)
                weak = T("S5")
                v.tensor_tensor(weak[:], weakish[:], strong[:], AL.subtract)
                # ---- hysteresis ----
                tp = T("S7")
                nc.gpsimd.dma_start(out=tp[0:119], in_=strong[1:120])
                tm = T("S8")
                nc.gpsimd.dma_start(out=tm[1:120], in_=strong[0:119])
                sd = T("S1")
                v.tensor_tensor(sd[:], tp[:], tm[:], AL.add)
                sh = T("S2")
                v.tensor_tensor(sh[:, 4:90, :], strong[:, 3:89, :], strong[:, 5:91, :],
                                AL.add)
                sw = T("S6")
                v.tensor_tensor(sw[:, :, 4:44], strong[:, :, 3:43], strong[:, :, 5:45],
                                AL.add)
                sa = T("S3")
                v.tensor_tensor(sa[:], sd[:], sh[:], AL.add)
                any6 = T("S1")
                v.tensor_tensor(any6[:], sa[:], sw[:], AL.add)
                wa = T("S2")
                v.scalar_tensor_tensor(wa[:], any6[:], 0.5, weak[:], AL.is_ge, AL.mult)
                of = T("S3")
                v.tensor_tensor(of[:], wa[:], strong[:], AL.max)
                # ---- bit-pack 8 w-voxels/byte (little-endian) ----
                pk1 = T("K1", cols=20)
                v.scalar_tensor_tensor(pk1[:, :, 0:20], of[:, :, 5:45:2], 2.0,
                                       of[:, :, 4:44:2], AL.mult, AL.add)
                pk2 = T("K2", cols=10)
                v.scalar_tensor_tensor(pk2[:, :, 0:10], pk1[:, :, 1:20:2], 4.0,
                                       pk1[:, :, 0:20:2], AL.mult, AL.add)
                pk3 = T("K3", cols=5, dt=U8)
                v.scalar_tensor_tensor(pk3[:, :, 0:5], pk2[:, :, 1:10:2], 16.0,
                                       pk2[:, :, 0:10:2], AL.mult, AL.add)

                ob = 5 if t < N_WT - 1 else 2
                for s in range(3):
                    r0, nr, h0 = STRIP_OUT[s]
                    nc.gpsimd.dma_start(
                        out=y[:, h0:h0 + nr, 5 * t:5 * t + ob],
                        in_=pk3[s * DLOC + 4:s * DLOC + 36, r0:r0 + nr, 0:ob],
                    )
    orig = nc.to_json_bytes
    nc.to_json_bytes = lambda: _fix_bir_json_bytes(orig())
    return nc


_NC_CACHE = None


def kernel(x: np.ndarray) -> np.ndarray:
    global _NC_CACHE
    x3 = np.asarray(x[0], dtype=np.float32)
    q = (x3 * np.float32(QF) + np.float32(0.5)).astype(np.uint16)
    s16 = (q ^ np.uint16(0x8000)).view(np.int16)

    in_maps = []
    for c in range(N_CORES):
        dmv = np.ones((NPART, 1), np.float32)
        dsv = np.zeros((NPART, 1), np.float32)
        if c == 0:
            dmv[[4, 44, 84]] = 0.0
            dsv[[4, 44, 84]] = U
        if c == N_CORES - 1:
            dmv[[35, 75, 115]] = 0.0
            dsv[[35, 75, 115]] = -U
        in_maps.append({"x": s16[DSH * c:DSH * (c + 1)], "dmask": dmv,
                        "dsel": dsv})

    if _NC_CACHE is None:
        _NC_CACHE = _build()
    res = run_bass_kernel_spmd(_NC_CACHE, in_maps, list(range(N_CORES)))
    yp = np.concatenate([r["y"] for r in res.results], axis=0)  # (256,256,32) u8
    bits = np.unpackbits(yp, axis=-1, bitorder="little")        # (256,256,256)
    return bits[None].view(np.int8)


# revision 8
# speedup vs baseline: 2.3209x; 1.0286x over previous
"""3D Canny edge detector on 8 Trainium2 cores.

Shard D=256 across 8 cores (32 slices each), *disjoint* host-side transfer
(the axon host<->device tunnel at ~45MB/s dominates wall time), with the
4-slice d-halo exchanged on device via an AllGather of each core's 8
boundary slices; neighbor blocks are selected with partition_id-derived
dynamic DMA offsets. A padded (40,264,264) slab is then assembled in
device DRAM: disjoint block + halos + h/w reflect fix-ups bounced through
SBUF (the 3-voxel zero margins are left uninitialized - only their
finiteness matters, every consumer is masked). The d-direction reflect at
the global d=0/255 borders cannot come from a neighbor, so it is applied
as a per-partition linear correction (dsel input, +-u at the border
partitions) to the smoothed volume: sm += dsel * (sp - sn) replaces the
missing/garbage smwh[d-1] term with smwh[d+1] exactly.

Input is quantized host-side to int16 (uint16 grid XOR 0x8000; the -32768
offset cancels exactly in the zero-sum Sobel kernels) and the binary
output is bit-packed on device to uint8 (8 voxels/byte along w,
little-endian), unpacked host-side with np.unpackbits. sqrt is eliminated
by comparing squared magnitudes against squared thresholds; the Gaussian
is applied unnormalized ([u,1,u] per axis) with normalization and the
65535 input scale folded into the thresholds. Per-core layout: partitions
= 3 h-strips x 40 local d-slices, free dim = (94 h-rows, 48 w-cols) per
w-tile; all three stencil axes are partition- or free-dim shifts.
"""
import json
import numpy as np

import concourse.bass as bass
import concourse.mybir as mybir
from concourse.bass_utils import run_bass_kernel_spmd
from concourse.tile import TileContext

F32 = mybir.dt.float32
I16 = mybir.dt.int16
U8 = mybir.dt.uint8
AL = mybir.AluOpType
SQ = mybir.ActivationFunctionType.Square
CP = mybir.ActivationFunctionType.Copy

N_CORES = 8
D, H, W = 256, 256, 256
DSH = 32            # disjoint d slices per core
DLOC = 40           # 32 output slices + 4 halo each side
NPART = 120         # 3 strips * 40
ROWS = 94           # h rows per strip tile (out rows + up to 4 halo each side)
COLS = 48           # w cols per tile (40 out + 4 halo each side)
WT_OUT = 40
N_WT = 7
STRIP_OFF = (0, 85, 170)                       # padded-h offset per strip
STRIP_OUT = ((4, 86, 0), (5, 85, 86), (5, 85, 171))  # (first r, n rows, h0)

U = float(np.exp(np.float64(-0.5)))
SC = (1.0 + 2.0 * U) ** 3
QF = 65535.0
HI2 = float((0.2 * SC * QF) ** 2)
LO2 = float((0.1 * SC * QF) ** 2)


def _fix_bir_json_bytes(raw: bytes) -> bytes:
    """walrus codegen has per-instruction sync-wait-slot limits (1 for CTRL
    Drain, 2 for compute structs). Hoist excess waits onto prepended
    single-wait Drain instructions on the same engine."""
    m = json.loads(raw)
    changed = False
    for fn in m.get("functions", []):
        for bb in fn.get("blocks", []):
            out = []
            for inst in bb.get("instructions", []):
                si = inst.get("sync_info") or {}
                waits = si.get("on_wait") or []
                lim = 1
                if len(waits) > lim and inst.get("engine") not in (None, "Unassigned"):
                    changed = True
                    keep_n = lim
                    for i, wt in enumerate(waits[:-keep_n] if keep_n else waits):
                        out.append({
                            "debug": inst.get("debug", 0),
                            "engine": inst["engine"],
                            "ins": [], "outs": [],
                            "is_reset_sema": False,
                            "name": f"{inst['name']}-w{i}",
                            "opcode": "Drain",
                            "sync_info": {"on_update": [], "on_wait": [wt]},
                        })
                    si["on_wait"] = waits[-keep_n:] if keep_n else []
                    inst["sync_info"] = si
                out.append(inst)
            bb["instructions"] = out
    return json.dumps(m).encode() if changed else raw


def _build():
    nc = bass.Bass("TRN2", target_bir_lowering=False, debug=False, num_devices=8)
    x = nc.dram_tensor("x", [DSH, 256, 256], I16, kind="ExternalInput").ap()
    dmask = nc.dram_tensor("dmask", [NPART, 1], F32, kind="ExternalInput").ap()
    dsel = nc.dram_tensor("dsel", [NPART, 1], F32, kind="ExternalInput").ap()
    y = nc.dram_tensor("y", [32, H, 32], U8, kind="ExternalOutput").ap()
    bnd = nc.dram_tensor("bnd", [8, 256, 256], I16, kind="Internal").ap()
    agb = nc.dram_tensor("agb", [64, 256, 256], I16, kind="Internal").ap()
    slab = nc.dram_tensor("slab", [DLOC, 264, 264], I16, kind="Internal").ap()

    _n = [0]

    def _ctr():
        _n[0] += 1
        return _n[0]

    with TileContext(nc) as tc:
        with tc.tile_pool(name="p", bufs=1) as pool:
            dm = pool.tile([NPART, 1], F32, tag="dm", name="dm0")
            nc.gpsimd.dma_start(out=dm[:], in_=dmask[:])
            dsl = pool.tile([NPART, 1], F32, tag="dsl", name="dsl0")
            nc.gpsimd.dma_start(out=dsl[:], in_=dsel[:])
            zrow = pool.tile([NPART, COLS], F32, tag="zr", name="zr0")
            nc.gpsimd.memset(zrow[:], 0.0)

            # ---- halo exchange: own boundary slices -> AllGather ----
            nc.gpsimd.dma_start(out=bnd[0:4], in_=x[0:4])
            nc.gpsimd.dma_start(out=bnd[4:8], in_=x[DSH - 4:DSH])
            nc.gpsimd.collective_compute(
                "AllGather", mybir.AluOpType.bypass,
                replica_groups=[list(range(N_CORES))],
                ins=[bnd[:].opt()], outs=[agb[:].opt()],
            )
            pid = nc.gpsimd.partition_id()
            top_off = ((pid + (N_CORES - 1)) % N_CORES) * 8 + 4
            bot_off = ((pid + 1) % N_CORES) * 8
            # ---- assemble padded slab in DRAM ----
            nc.gpsimd.dma_start(out=slab[0:4, 4:260, 4:260],
                                in_=agb[bass.ds(top_off, 4)])
            nc.gpsimd.dma_start(out=slab[4:36, 4:260, 4:260], in_=x[:])
            nc.gpsimd.dma_start(out=slab[36:40, 4:260, 4:260],
                                in_=agb[bass.ds(bot_off, 4)])
            # h/w reflect fix-ups are applied in SBUF on the loaded tiles
            # below (staged like the d-shift copies); slab's h/w margin
            # rows/cols 0:4 and 260:264 stay uninitialized - only their
            # finiteness matters, every consumer is masked or overwritten.

            for t in range(N_WT):
                c0 = WT_OUT * t
                in_w = min(COLS, 264 - c0)

                def T(tag, cols=COLS, dt=F32):
                    return pool.tile([NPART, ROWS, cols], dt, tag=tag,
                                     name=f"{tag}_{t}_{_ctr()}")

                v = nc.vector
                xu = T("S9", dt=I16)
                for s in range(3):
                    nc.gpsimd.dma_start(
                        out=xu[s * DLOC:(s + 1) * DLOC, :, 0:in_w],
                        in_=slab[:, STRIP_OFF[s]:STRIP_OFF[s] + ROWS, c0:c0 + in_w],
                    )
                # h reflect: strip 0 padded row 3 <- raw row 1 (tile row 5);
                # strip 2 padded row 260 (tile row 90) <- raw 254 (row 88).
                rf = pool.tile([NPART, 1, COLS], I16, tag="RF",
                               name=f"rf_{t}")
                nc.gpsimd.dma_start(out=rf[0:40, 0:1, :], in_=xu[0:40, 5:6, :])
                nc.gpsimd.dma_start(out=xu[0:40, 3:4, :], in_=rf[0:40, 0:1, :])
                nc.gpsimd.dma_start(out=rf[80:120, 0:1, :], in_=xu[80:120, 88:89, :])
                nc.gpsimd.dma_start(out=xu[80:120, 90:91, :], in_=rf[80:120, 0:1, :])
                # w reflect (after rows, so corners inherit the row fix):
                # t=0 padded col 3 <- raw col 1 (col 5); t=6 padded col 260
                # (col 20) <- raw col 254 (col 18).
                if t == 0 or t == N_WT - 1:
                    csrc, cdst = (5, 3) if t == 0 else (18, 20)
                    cf = pool.tile([NPART, ROWS, 1], I16, tag="CF",
                                   name=f"cf_{t}")
                    nc.gpsimd.dma_start(out=cf[:], in_=xu[:, :, csrc:csrc + 1])
                    nc.gpsimd.dma_start(out=xu[:, :, cdst:cdst + 1], in_=cf[:])
                xt = T("S1")
                nc.scalar.copy(xt[:], xu[:])
                # ---- Gaussian [u,1,u] along w, h, d ----
                tw = T("S2")
                v.tensor_tensor(tw[:, :, 1:47], xt[:, :, 0:46], xt[:, :, 2:48], AL.add)
                smw = T("S3")
                v.scalar_tensor_tensor(smw[:, :, 1:47], tw[:, :, 1:47], U,
                                       xt[:, :, 1:47], AL.mult, AL.add)
                th = T("S2")
                v.tensor_tensor(th[:, 1:93, :], smw[:, 0:92, :], smw[:, 2:94, :], AL.add)
                smwh = T("S1")
                v.scalar_tensor_tensor(smwh[:, 1:93, :], th[:, 1:93, :], U,
                                       smw[:, 1:93, :], AL.mult, AL.add)
                # d-shift staging copies (DMA partition realign; compute stays
                # at partition start 0 per ISA 32-alignment rule)
                sp = T("S7")
                nc.gpsimd.dma_start(out=sp[0:119], in_=smwh[1:120])
                sn = T("S8")
                nc.gpsimd.dma_start(out=sn[1:120], in_=smwh[0:119])
                td = T("S2")
                v.tensor_tensor(td[:], sn[:], sp[:], AL.add)
                sm0 = T("S3")
                v.scalar_tensor_tensor(sm0[:], td[:], U, smwh[:], AL.mult, AL.add)
                # d-reflect correction at global d borders: sm += dsel*(sp-sn)
                diff = T("S2")
                v.tensor_tensor(diff[:], sp[:], sn[:], AL.subtract)
                fixt = T("S1")
                nc.scalar.activation(fixt[:], diff[:], CP, scale=dsl[:, 0:1])
                sm = T("S2")
                v.tensor_tensor(sm[:], sm0[:], fixt[:], AL.add)
                # ---- Sobel d-stage: A = sm*[1,1,1]_d, B = sm*[-1,0,1]_d ----
                p2 = T("S7")
                nc.gpsimd.dma_start(out=p2[0:119], in_=sm[1:120])
                m2 = T("S8")
                nc.gpsimd.dma_start(out=m2[1:120], in_=sm[0:119])
                a1 = T("S1")
                v.tensor_tensor(a1[:], p2[:], m2[:], AL.add)
                A = T("S3")
                v.tensor_tensor(A[:], a1[:], sm[:], AL.add)
                B = T("S2")
                v.tensor_tensor(B[:], p2[:], m2[:], AL.subtract)
                # ---- gx = A *h [1,2,1] *w [-1,0,1] ----
                ph = T("S1")
                v.tensor_tensor(ph[:, 2:92, :], A[:, 1:91, :], A[:, 3:93, :], AL.add)
                gxh = T("S4")
                v.scalar_tensor_tensor(gxh[:, 2:92, :], A[:, 2:92, :], 2.0,
                                       ph[:, 2:92, :], AL.mult, AL.add)
                gx = T("S1")
                v.tensor_tensor(gx[:, :, 2:46], gxh[:, :, 3:47], gxh[:, :, 1:45],
                                AL.subtract)
                # ---- gy = A *h [-1,0,1] *w [1,2,1] ----
                gyh = T("S5")
                v.tensor_tensor(gyh[:, 2:92, :], A[:, 3:93, :], A[:, 1:91, :],
                                AL.subtract)
                pw = T("S6")
                v.tensor_tensor(pw[:, :, 2:46], gyh[:, :, 1:45], gyh[:, :, 3:47], AL.add)
                gy = T("S4")
                v.scalar_tensor_tensor(gy[:, :, 2:46], gyh[:, :, 2:46], 2.0,
                                       pw[:, :, 2:46], AL.mult, AL.add)
                # ---- gz = B *h [1,1,1] *w [1,1,1] ----
                bh1 = T("S7")
                v.tensor_tensor(bh1[:, 2:92, :], B[:, 1:91, :], B[:, 3:93, :], AL.add)
                bh = T("S8")
                v.tensor_tensor(bh[:, 2:92, :], bh1[:, 2:92, :], B[:, 2:92, :], AL.add)
                bw1 = T("S5")
                v.tensor_tensor(bw1[:, :, 2:46], bh[:, :, 1:45], bh[:, :, 3:47], AL.add)
                gz = T("S2")
                v.tensor_tensor(gz[:, :, 2:46], bw1[:, :, 2:46], bh[:, :, 2:46], AL.add)
                # ---- msq = dmask*(gx^2+gy^2+gz^2), then h/w border zeroing ----
                sx = T("S7")
                nc.scalar.activation(sx[:], gx[:], SQ, scale=dm[:, 0:1])
                sy = T("S1")
                nc.scalar.activation(sy[:], gy[:], SQ, scale=dm[:, 0:1])
                sz = T("S5")
                nc.scalar.activation(sz[:], gz[:], SQ, scale=dm[:, 0:1])
                m1 = T("S2")
                v.tensor_tensor(m1[:], sx[:], sy[:], AL.add)
                msq = T("S4")
                v.tensor_tensor(msq[:], m1[:], sz[:], AL.add)
                nc.gpsimd.dma_start(out=msq[0:40, 4:5, :], in_=zrow[0:40, :])
                nc.gpsimd.dma_start(out=msq[80:120, 89:90, :], in_=zrow[80:120, :])
                if t == 0:
                    nc.gpsimd.memset(msq[:, :, 4:5], 0.0)
                if t == N_WT - 1:
                    nc.gpsimd.memset(msq[:, :, 19:20], 0.0)
                # ---- NMS ----
                r2 = T("S1")
                v.tensor_tensor(r2[:, :, 3:45], msq[:, :, 2:44], msq[:, :, 4:46], AL.max)
                r3 = T("S5")
                v.tensor_tensor(r3[:, :, 3:45], r2[:, :, 3:45], msq[:, :, 3:45], AL.max)
                mh = T("S2")
                v.tensor_tensor(mh[:, 3:91, :], r3[:, 2:90, :], r3[:, 4:92, :], AL.max)
                nb8 = T("S5")
                v.tensor_tensor(nb8[:, 3:91, :], mh[:, 3:91, :], r2[:, 3:91, :], AL.max)
                nbm = T("S7")
                nc.gpsimd.dma_start(out=nbm[1:120], in_=nb8[0:119])
                keep = T("S1")
                v.tensor_tensor(keep[:], msq[:], nbm[:], AL.is_gt)
                nmsq = T("S2")
                v.tensor_tensor(nmsq[:], msq[:], keep[:], AL.mult)
                # ---- thresholds ----
                strong = T("S4")
                v.tensor_scalar(strong[:], nmsq[:], HI2, None, AL.is_gt)
                weakish = T("S1")
                v.tensor_scalar(weakish[:], nmsq[:], LO2, None, AL.is_gt)
                weak = T("S5")
                v.tensor_tensor(weak[:], weakish[:], strong[:], AL.subtract)
                # ---- hysteresis ----
                tp = T("S7")
                nc.gpsimd.dma_start(out=tp[0:119], in_=strong[1:120])
                tm = T("S8")
                nc.gpsimd.dma_start(out=tm[1:120], in_=strong[0:119])
                sd = T("S1")
                v.tensor_tensor(sd[:], tp[:], tm[:], AL.add)
                sh = T("S2")
                v.tensor_tensor(sh[:, 4:90, :], strong[:, 3:89, :], strong[:, 5:91, :],
                                AL.add)
                sw = T("S6")
                v.tensor_tensor(sw[:, :, 4:44], strong[:, :, 3:43], strong[:, :, 5:45],
                                AL.add)
                sa = T("S3")
                v.tensor_tensor(sa[:], sd[:], sh[:], AL.add)
                any6 = T("S1")
                v.tensor_tensor(any6[:], sa[:], sw[:], AL.add)
                wa = T("S2")
                v.scalar_tensor_tensor(wa[:], any6[:], 0.5, weak[:], AL.is_ge, AL.mult)
                of = T("S3")
                v.tensor_tensor(of[:], wa[:], strong[:], AL.max)
                # ---- bit-pack 8 w-voxels/byte (little-endian) ----
                pk1 = T("K1", cols=20)
                v.scalar_tensor_tensor(pk1[:, :, 0:20], of[:, :, 5:45:2], 2.0,
                                       of[:, :, 4:44:2], AL.mult, AL.add)
                pk2 = T("K2", cols=10)
                v.scalar_tensor_tensor(pk2[:, :, 0:10], pk1[:, :, 1:20:2], 4.0,
                                       pk1[:, :, 0:20:2], AL.mult, AL.add)
                pk3 = T("K3", cols=5, dt=U8)
                v.scalar_tensor_tensor(pk3[:, :, 0:5], pk2[:, :, 1:10:2], 16.0,
                                       pk2[:, :, 0:10:2], AL.mult, AL.add)

                ob = 5 if t < N_WT - 1 else 2
                for s in range(3):
                    r0, nr, h0 = STRIP_OUT[s]
                    nc.gpsimd.dma_start(
                        out=y[:, h0:h0 + nr, 5 * t:5 * t + ob],
                        in_=pk3[s * DLOC + 4:s * DLOC + 36, r0:r0 + nr, 0:ob],
                    )
    orig = nc.to_json_bytes
    nc.to_json_bytes = lambda: _fix_bir_json_bytes(orig())
    return nc


_NC_CACHE = None


def kernel(x: np.ndarray) -> np.ndarray:
    global _NC_CACHE
    from concurrent.futures import ThreadPoolExecutor
    x3 = np.asarray(x[0], dtype=np.float32)
    s16 = np.empty((D, H, W), np.int16)

    def _qchunk(c):
        a = x3[DSH * c:DSH * (c + 1)]
        q = (a * np.float32(QF) + np.float32(0.5)).astype(np.uint16)
        s16[DSH * c:DSH * (c + 1)] = (q ^ np.uint16(0x8000)).view(np.int16)

    with ThreadPoolExecutor(8) as ex:
        list(ex.map(_qchunk, range(N_CORES)))

    in_maps = []
    for c in range(N_CORES):
        dmv = np.ones((NPART, 1), np.float32)
        dsv = np.zeros((NPART, 1), np.float32)
        if c == 0:
            dmv[[4, 44, 84]] = 0.0
            dsv[[4, 44, 84]] = U
        if c == N_CORES - 1:
            dmv[[35, 75, 115]] = 0.0
            dsv[[35, 75, 115]] = -U
        in_maps.append({"x": s16[DSH * c:DSH * (c + 1)], "dmask": dmv,
                        "dsel": dsv})

    if _NC_CACHE is None:
        _NC_CACHE = _build()
    res = run_bass_kernel_spmd(_NC_CACHE, in_maps, list(range(N_CORES)))
    yp = np.concatenate([r["y"] for r in res.results], axis=0)  # (256,256,32) u8
    bits = np.unpackbits(yp, axis=-1, bitorder="little")        # (256,256,256)
    return bits[None].view(np.int8)


# revision 9
# speedup vs baseline: 2.4018x; 1.0348x over previous
"""3D Canny edge detector on 8 Trainium2 cores.

Shard D=256 across 8 cores (32 slices each), *disjoint* host-side transfer
(the axon host<->device tunnel at ~45MB/s dominates wall time), with the
4-slice d-halo exchanged on device via an AllGather of each core's 8
boundary slices; neighbor blocks are selected with partition_id-derived
dynamic DMA offsets. A padded (40,264,264) slab is then assembled in
device DRAM: disjoint block + halos + h/w reflect fix-ups bounced through
SBUF (the 3-voxel zero margins are left uninitialized - only their
finiteness matters, every consumer is masked). The d-direction reflect at
the global d=0/255 borders cannot come from a neighbor, so it is applied
as a per-partition linear correction (dsel input, +-u at the border
partitions) to the smoothed volume: sm += dsel * (sp - sn) replaces the
missing/garbage smwh[d-1] term with smwh[d+1] exactly.

Input is quantized host-side to int16 (uint16 grid XOR 0x8000; the -32768
offset cancels exactly in the zero-sum Sobel kernels) and the binary
output is bit-packed on device to uint8 (8 voxels/byte along w,
little-endian), unpacked host-side with np.unpackbits. sqrt is eliminated
by comparing squared magnitudes against squared thresholds; the Gaussian
is applied unnormalized ([u,1,u] per axis) with normalization and the
65535 input scale folded into the thresholds. Per-core layout: partitions
= 3 h-strips x 40 local d-slices, free dim = (94 h-rows, 48 w-cols) per
w-tile; all three stencil axes are partition- or free-dim shifts.
"""
import json
import numpy as np

import concourse.bass as bass
import concourse.mybir as mybir
from concourse.bass_utils import run_bass_kernel_spmd
from concourse.tile import TileContext

F32 = mybir.dt.float32
I16 = mybir.dt.int16
U8 = mybir.dt.uint8
AL = mybir.AluOpType
SQ = mybir.ActivationFunctionType.Square
CP = mybir.ActivationFunctionType.Copy

N_CORES = 8
D, H, W = 256, 256, 256
DSH = 32            # disjoint d slices per core
DLOC = 40           # 32 output slices + 4 halo each side
NPART = 120         # 3 strips * 40
ROWS = 94           # h rows per strip tile (out rows + up to 4 halo each side)
COLS = 48           # w cols per tile (40 out + 4 halo each side)
WT_OUT = 40
N_WT = 7
STRIP_OFF = (0, 85, 170)                       # padded-h offset per strip
STRIP_OUT = ((4, 86, 0), (5, 85, 86), (5, 85, 171))  # (first r, n rows, h0)

U = float(np.exp(np.float64(-0.5)))
SC = (1.0 + 2.0 * U) ** 3
QF = 65535.0
HI2 = float((0.2 * SC * QF) ** 2)
LO2 = float((0.1 * SC * QF) ** 2)


def _fix_bir_json_bytes(raw: bytes) -> bytes:
    """walrus codegen has per-instruction sync-wait-slot limits (1 for CTRL
    Drain, 2 for compute structs). Hoist excess waits onto prepended
    single-wait Drain instructions on the same engine."""
    m = json.loads(raw)
    changed = False
    for fn in m.get("functions", []):
        for bb in fn.get("blocks", []):
            out = []
            for inst in bb.get("instructions", []):
                si = inst.get("sync_info") or {}
                waits = si.get("on_wait") or []
                lim = 1
                if len(waits) > lim and inst.get("engine") not in (None, "Unassigned"):
                    changed = True
                    keep_n = lim
                    for i, wt in enumerate(waits[:-keep_n] if keep_n else waits):
                        out.append({
                            "debug": inst.get("debug", 0),
                            "engine": inst["engine"],
                            "ins": [], "outs": [],
                            "is_reset_sema": False,
                            "name": f"{inst['name']}-w{i}",
                            "opcode": "Drain",
                            "sync_info": {"on_update": [], "on_wait": [wt]},
                        })
                    si["on_wait"] = waits[-keep_n:] if keep_n else []
                    inst["sync_info"] = si
                out.append(inst)
            bb["instructions"] = out
    return json.dumps(m).encode() if changed else raw


def _build():
    nc = bass.Bass("TRN2", target_bir_lowering=False, debug=False, num_devices=8)
    x = nc.dram_tensor("x", [DSH, 256, 256], I16, kind="ExternalInput").ap()
    dmask = nc.dram_tensor("dmask", [NPART, 1], F32, kind="ExternalInput").ap()
    dsel = nc.dram_tensor("dsel", [NPART, 1], F32, kind="ExternalInput").ap()
    y = nc.dram_tensor("y", [32, H, 32], U8, kind="ExternalOutput").ap()
    bnd = nc.dram_tensor("bnd", [8, 256, 256], I16, kind="Internal").ap()
    agb = nc.dram_tensor("agb", [64, 256, 256], I16, kind="Internal").ap()
    slab = nc.dram_tensor("slab", [DLOC, 264, 264], I16, kind="Internal").ap()

    _n = [0]

    def _ctr():
        _n[0] += 1
        return _n[0]

    with TileContext(nc) as tc:
        with tc.tile_pool(name="p", bufs=1) as pool:
            dm = pool.tile([NPART, 1], F32, tag="dm", name="dm0")
            nc.gpsimd.dma_start(out=dm[:], in_=dmask[:])
            dsl = pool.tile([NPART, 1], F32, tag="dsl", name="dsl0")
            nc.gpsimd.dma_start(out=dsl[:], in_=dsel[:])
            zrow = pool.tile([NPART, COLS], F32, tag="zr", name="zr0")
            nc.gpsimd.memset(zrow[:], 0.0)

            # ---- halo exchange: own boundary slices -> AllGather ----
            nc.gpsimd.dma_start(out=bnd[0:4], in_=x[0:4])
            nc.gpsimd.dma_start(out=bnd[4:8], in_=x[DSH - 4:DSH])
            nc.gpsimd.collective_compute(
                "AllGather", mybir.AluOpType.bypass,
                replica_groups=[list(range(N_CORES))],
                ins=[bnd[:].opt()], outs=[agb[:].opt()],
            )
            pid = nc.gpsimd.partition_id()
            top_off = ((pid + (N_CORES - 1)) % N_CORES) * 8 + 4
            bot_off = ((pid + 1) % N_CORES) * 8
            # ---- assemble padded slab in DRAM ----
            nc.gpsimd.dma_start(out=slab[0:4, 4:260, 4:260],
                                in_=agb[bass.ds(top_off, 4)])
            nc.gpsimd.dma_start(out=slab[4:36, 4:260, 4:260], in_=x[:])
            nc.gpsimd.dma_start(out=slab[36:40, 4:260, 4:260],
                                in_=agb[bass.ds(bot_off, 4)])
            # h/w reflect fix-ups are applied in SBUF on the loaded tiles
            # below (staged like the d-shift copies); slab's h/w margin
            # rows/cols 0:4 and 260:264 stay uninitialized - only their
            # finiteness matters, every consumer is masked or overwritten.

            for t in range(N_WT):
                c0 = WT_OUT * t
                in_w = min(COLS, 264 - c0)

                def T(tag, cols=COLS, dt=F32):
                    return pool.tile([NPART, ROWS, cols], dt, tag=tag,
                                     name=f"{tag}_{t}_{_ctr()}")

                v = nc.vector
                xu = T("S9", dt=I16)
                for s in range(3):
                    nc.gpsimd.dma_start(
                        out=xu[s * DLOC:(s + 1) * DLOC, :, 0:in_w],
                        in_=slab[:, STRIP_OFF[s]:STRIP_OFF[s] + ROWS, c0:c0 + in_w],
                    )
                # h reflect: strip 0 padded row 3 <- raw row 1 (tile row 5);
                # strip 2 padded row 260 (tile row 90) <- raw 254 (row 88).
                rf = pool.tile([NPART, 1, COLS], I16, tag="RF",
                               name=f"rf_{t}")
                nc.gpsimd.dma_start(out=rf[0:40, 0:1, :], in_=xu[0:40, 5:6, :])
                nc.gpsimd.dma_start(out=xu[0:40, 3:4, :], in_=rf[0:40, 0:1, :])
                nc.gpsimd.dma_start(out=rf[80:120, 0:1, :], in_=xu[80:120, 88:89, :])
                nc.gpsimd.dma_start(out=xu[80:120, 90:91, :], in_=rf[80:120, 0:1, :])
                # w reflect (after rows, so corners inherit the row fix):
                # t=0 padded col 3 <- raw col 1 (col 5); t=6 padded col 260
                # (col 20) <- raw col 254 (col 18).
                if t == 0 or t == N_WT - 1:
                    csrc, cdst = (5, 3) if t == 0 else (18, 20)
                    cf = pool.tile([NPART, ROWS, 1], I16, tag="CF",
                                   name=f"cf_{t}")
                    nc.gpsimd.dma_start(out=cf[:], in_=xu[:, :, csrc:csrc + 1])
                    nc.gpsimd.dma_start(out=xu[:, :, cdst:cdst + 1], in_=cf[:])
                xt = T("S1")
                nc.scalar.copy(xt[:], xu[:])
                # ---- Gaussian [u,1,u] along w, h, d ----
                tw = T("S2")
                v.tensor_tensor(tw[:, :, 1:47], xt[:, :, 0:46], xt[:, :, 2:48], AL.add)
                smw = T("S3")
                v.scalar_tensor_tensor(smw[:, :, 1:47], tw[:, :, 1:47], U,
                                       xt[:, :, 1:47], AL.mult, AL.add)
                th = T("S2")
                v.tensor_tensor(th[:, 1:93, :], smw[:, 0:92, :], smw[:, 2:94, :], AL.add)
                smwh = T("S1")
                v.scalar_tensor_tensor(smwh[:, 1:93, :], th[:, 1:93, :], U,
                                       smw[:, 1:93, :], AL.mult, AL.add)
                # d-shift staging copies (DMA partition realign; compute stays
                # at partition start 0 per ISA 32-alignment rule)
                sp = T("S7")
                nc.gpsimd.dma_start(out=sp[0:119], in_=smwh[1:120])
                sn = T("S8")
                nc.gpsimd.dma_start(out=sn[1:120], in_=smwh[0:119])
                td = T("S2")
                v.tensor_tensor(td[:], sn[:], sp[:], AL.add)
                sm0 = T("S3")
                v.scalar_tensor_tensor(sm0[:], td[:], U, smwh[:], AL.mult, AL.add)
                # d-reflect correction at global d borders: sm += dsel*(sp-sn)
                diff = T("S2")
                v.tensor_tensor(diff[:], sp[:], sn[:], AL.subtract)
                fixt = T("S1")
                nc.scalar.activation(fixt[:], diff[:], CP, scale=dsl[:, 0:1])
                sm = T("S2")
                v.tensor_tensor(sm[:], sm0[:], fixt[:], AL.add)
                # ---- Sobel d-stage: A = sm*[1,1,1]_d, B = sm*[-1,0,1]_d ----
                p2 = T("S7")
                nc.gpsimd.dma_start(out=p2[0:119], in_=sm[1:120])
                m2 = T("S8")
                nc.gpsimd.dma_start(out=m2[1:120], in_=sm[0:119])
                a1 = T("S1")
                v.tensor_tensor(a1[:], p2[:], m2[:], AL.add)
                A = T("S3")
                v.tensor_tensor(A[:], a1[:], sm[:], AL.add)
                B = T("S2")
                v.tensor_tensor(B[:], p2[:], m2[:], AL.subtract)
                # ---- gx = A *h [1,2,1] *w [-1,0,1] ----
                ph = T("S1")
                v.tensor_tensor(ph[:, 2:92, :], A[:, 1:91, :], A[:, 3:93, :], AL.add)
                gxh = T("S4")
                v.scalar_tensor_tensor(gxh[:, 2:92, :], A[:, 2:92, :], 2.0,
                                       ph[:, 2:92, :], AL.mult, AL.add)
                gx = T("S1")
                v.tensor_tensor(gx[:, :, 2:46], gxh[:, :, 3:47], gxh[:, :, 1:45],
                                AL.subtract)
                # ---- gy = A *h [-1,0,1] *w [1,2,1] ----
                gyh = T("S5")
                v.tensor_tensor(gyh[:, 2:92, :], A[:, 3:93, :], A[:, 1:91, :],
                                AL.subtract)
                pw = T("S6")
                v.tensor_tensor(pw[:, :, 2:46], gyh[:, :, 1:45], gyh[:, :, 3:47], AL.add)
                gy = T("S4")
                v.scalar_tensor_tensor(gy[:, :, 2:46], gyh[:, :, 2:46], 2.0,
                                       pw[:, :, 2:46], AL.mult, AL.add)
                # ---- gz = B *h [1,1,1] *w [1,1,1] ----
                bh1 = T("S7")
                v.tensor_tensor(bh1[:, 2:92, :], B[:, 1:91, :], B[:, 3:93, :], AL.add)
                bh = T("S8")
                v.tensor_tensor(bh[:, 2:92, :], bh1[:, 2:92, :], B[:, 2:92, :], AL.add)
                bw1 = T("S5")
                v.tensor_tensor(bw1[:, :, 2:46], bh[:, :, 1:45], bh[:, :, 3:47], AL.add)
                gz = T("S2")
                v.tensor_tensor(gz[:, :, 2:46], bw1[:, :, 2:46], bh[:, :, 2:46], AL.add)
                # ---- msq = dmask*(gx^2+gy^2+gz^2), then h/w border zeroing ----
                sx = T("S7")
                nc.scalar.activation(sx[:], gx[:], SQ, scale=dm[:, 0:1])
                sy = T("S1")
                nc.scalar.activation(sy[:], gy[:], SQ, scale=dm[:, 0:1])
                sz = T("S5")
                nc.scalar.activation(sz[:], gz[:], SQ, scale=dm[:, 0:1])
                m1 = T("S2")
                v.tensor_tensor(m1[:], sx[:], sy[:], AL.add)
                msq = T("S4")
                v.tensor_tensor(msq[:], m1[:], sz[:], AL.add)
                nc.gpsimd.dma_start(out=msq[0:40, 4:5, :], in_=zrow[0:40, :])
                nc.gpsimd.dma_start(out=msq[80:120, 89:90, :], in_=zrow[80:120, :])
                if t == 0:
                    nc.gpsimd.memset(msq[:, :, 4:5], 0.0)
                if t == N_WT - 1:
                    nc.gpsimd.memset(msq[:, :, 19:20], 0.0)
                # ---- NMS ----
                r2 = T("S1")
                v.tensor_tensor(r2[:, :, 3:45], msq[:, :, 2:44], msq[:, :, 4:46], AL.max)
                r3 = T("S5")
                v.tensor_tensor(r3[:, :, 3:45], r2[:, :, 3:45], msq[:, :, 3:45], AL.max)
                mh = T("S2")
                v.tensor_tensor(mh[:, 3:91, :], r3[:, 2:90, :], r3[:, 4:92, :], AL.max)
                nb8 = T("S5")
                v.tensor_tensor(nb8[:, 3:91, :], mh[:, 3:91, :], r2[:, 3:91, :], AL.max)
                nbm = T("S7")
                nc.gpsimd.dma_start(out=nbm[1:120], in_=nb8[0:119])
                keep = T("S1")
                v.tensor_tensor(keep[:], msq[:], nbm[:], AL.is_gt)
                nmsq = T("S2")
                v.tensor_tensor(nmsq[:], msq[:], keep[:], AL.mult)
                # ---- thresholds ----
                strong = T("S4")
                v.tensor_scalar(strong[:], nmsq[:], HI2, None, AL.is_gt)
                weakish = T("S1")
                v.tensor_scalar(weakish[:], nmsq[:], LO2, None, AL.is_gt)
                weak = T("S5")
                v.tensor_tensor(weak[:], weakish[:], strong[:], AL.subtract)
                # ---- hysteresis ----
                tp = T("S7")
                nc.gpsimd.dma_start(out=tp[0:119], in_=strong[1:120])
                tm = T("S8")
                nc.gpsimd.dma_start(out=tm[1:120], in_=strong[0:119])
                sd = T("S1")
                v.tensor_tensor(sd[:], tp[:], tm[:], AL.add)
                sh = T("S2")
                v.tensor_tensor(sh[:, 4:90, :], strong[:, 3:89, :], strong[:, 5:91, :],
                                AL.add)
                sw = T("S6")
                v.tensor_tensor(sw[:, :, 4:44], strong[:, :, 3:43], strong[:, :, 5:45],
                                AL.add)
                sa = T("S3")
                v.tensor_tensor(sa[:], sd[:], sh[:], AL.add)
                any6 = T("S1")
                v.tensor_tensor(any6[:], sa[:], sw[:], AL.add)
                wa = T("S2")
                v.scalar_tensor_tensor(wa[:], any6[:], 0.5, weak[:], AL.is_ge, AL.mult)
                of = T("S3")
                v.tensor_tensor(of[:], wa[:], strong[:], AL.max)
                # ---- bit-pack 8 w-voxels/byte (little-endian) ----
                pk1 = T("K1", cols=20)
                v.scalar_tensor_tensor(pk1[:, :, 0:20], of[:, :, 5:45:2], 2.0,
                                       of[:, :, 4:44:2], AL.mult, AL.add)
                pk2 = T("K2", cols=10)
                v.scalar_tensor_tensor(pk2[:, :, 0:10], pk1[:, :, 1:20:2], 4.0,
                                       pk1[:, :, 0:20:2], AL.mult, AL.add)
                pk3 = T("K3", cols=5, dt=U8)
                v.scalar_tensor_tensor(pk3[:, :, 0:5], pk2[:, :, 1:10:2], 16.0,
                                       pk2[:, :, 0:10:2], AL.mult, AL.add)

                ob = 5 if t < N_WT - 1 else 2
                for s in range(3):
                    r0, nr, h0 = STRIP_OUT[s]
                    nc.gpsimd.dma_start(
                        out=y[:, h0:h0 + nr, 5 * t:5 * t + ob],
                        in_=pk3[s * DLOC + 4:s * DLOC + 36, r0:r0 + nr, 0:ob],
                    )
    orig = nc.to_json_bytes
    nc.to_json_bytes = lambda: _fix_bir_json_bytes(orig())
    return nc


_NC_CACHE = None
_POOL = None
_MASKS = None


def _static_masks():
    global _MASKS
    if _MASKS is None:
        masks = []
        for c in range(N_CORES):
            dmv = np.ones((NPART, 1), np.float32)
            dsv = np.zeros((NPART, 1), np.float32)
            if c == 0:
                dmv[[4, 44, 84]] = 0.0
                dsv[[4, 44, 84]] = U
            if c == N_CORES - 1:
                dmv[[35, 75, 115]] = 0.0
                dsv[[35, 75, 115]] = -U
            masks.append((dmv, dsv))
        _MASKS = masks
    return _MASKS


def kernel(x: np.ndarray) -> np.ndarray:
    global _NC_CACHE, _POOL
    from concurrent.futures import ThreadPoolExecutor
    if _POOL is None:
        _POOL = ThreadPoolExecutor(8)
    x3 = np.asarray(x[0], dtype=np.float32)
    s16 = np.empty((D, H, W), np.int16)

    def _qchunk(c):
        a = x3[DSH * c:DSH * (c + 1)]
        q = (a * np.float32(QF) + np.float32(0.5)).astype(np.uint16)
        s16[DSH * c:DSH * (c + 1)] = (q ^ np.uint16(0x8000)).view(np.int16)

    list(_POOL.map(_qchunk, range(N_CORES)))

    in_maps = [{"x": s16[DSH * c:DSH * (c + 1)], "dmask": dmv, "dsel": dsv}
               for c, (dmv, dsv) in enumerate(_static_masks())]

    if _NC_CACHE is None:
        _NC_CACHE = _build()
    res = run_bass_kernel_spmd(_NC_CACHE, in_maps, list(range(N_CORES)))
    yp = np.concatenate([r["y"] for r in res.results], axis=0)  # (256,256,32) u8
    bits = np.unpackbits(yp, axis=-1, bitorder="little")        # (256,256,256)
    return bits[None].view(np.int8)
